# revision 39
# baseline (speedup 1.0000x reference)
"""Bahdanau attention cell (location-sensitive) on 8 TRN2 NeuronCores.

Sharding: data-parallel over the batch dim (64 -> 8 batches/core); all
params (conv kernel, location dense, score v/b) are tiny and replicated.

Per-core device program (Bass/Tile), [t-on-partitions, a-on-free] layout:
  1. conv(prev_weights) as a block-diagonal bf16 matmul over an im2col
     matrix (gpsimd cast-DMA); evacuation bias fabricates constant-1 rows
     so the location matmul can fold in query+score_b (split into two
     bf16-exact rows).
  2. main pass per (batch, 512-t group): PSUM[t, a] = ident16 @ w_bf16
     (w accumulate) + f-chunk.T @ [loc_w ; q_hi ; q_lo]; tanh on ACT ->
     fp16; DVE mult by score_v + segmented reduce -> e_cols[toff, b*16+j].
  3. softmax entirely in the column domain ([toff, (b,j)] tiles, tiny
     ops; cross-partition max/sum via gpsimd partition_all_reduce), one
     PE transpose, stream-matched DMAs straight to the padded outputs.
"""

import sys

sys.path.insert(0, "/opt/trn_rl_repo")

import numpy as np

import concourse.bacc as bacc
import concourse.bass as bass
import concourse.bass_isa as bass_isa
import concourse.tile as tile
from concourse import mybir
from concourse.bass_utils import run_bass_kernel_spmd

B, T, A, F, KW = 64, 2000, 256, 32, 31
NCORES = 8
BL = B // NCORES  # 8 batches per core
PAD = (KW - 1) // 2  # 15
TP = T + 2 * PAD
TC = 2048  # t padded to 16 chunks of 128
TP2 = TC + 2 * PAD  # padded prev: conv runs over all TC columns
NCH = TC // 128  # 16 chunks per batch
NG = 4  # groups of 4 chunks (512 t) per batch
F32 = mybir.dt.float32
F16 = mybir.dt.float16
BF16 = mybir.dt.bfloat16

TR = [(0, 1024), (1024, 1024)]
NGP_MOD = 0  # gpsimd cannot access PSUM on TRN2: keep w-add on PE


def build_program():
    nc = bacc.Bacc("TRN2", target_bir_lowering=False)

    wmem = nc.dram_tensor("wmem", [BL // 2, TC, A], F32, kind="ExternalInput")
    wmem2 = nc.dram_tensor("wmem2", [BL // 2, TC, A], mybir.dt.float32r,
                           kind="ExternalInput")
    prevp = nc.dram_tensor("prevp", [BL, TP2], F32, kind="ExternalInput")
    locq = nc.dram_tensor("locq", [98, BL * A], BF16, kind="ExternalInput")
    bd = nc.dram_tensor("bd", [2 * KW, 98], BF16, kind="ExternalInput")
    cb66 = nc.dram_tensor("cb66", [98, 1], F32, kind="ExternalInput")
    vrep = nc.dram_tensor("vrep", [128, 4 * A], F16, kind="ExternalInput")
    ident = nc.dram_tensor("ident", [128, 128], F32, kind="ExternalInput")
    ident_h = nc.dram_tensor("ident_h", [128, 128], BF16, kind="ExternalInput")
    maskc = nc.dram_tensor("maskc", [128, 128], F32, kind="ExternalInput")
    prevc = nc.dram_tensor("prevc", [128, 128], F32, kind="ExternalInput")
    out_w = nc.dram_tensor("out_w", [BL, TC], F32, kind="ExternalOutput")
    out_nw = nc.dram_tensor("out_nw", [BL, TC], F32, kind="ExternalOutput")

    with tile.TileContext(nc) as tc:
        with (
            tc.tile_pool(name="singles", bufs=1) as singles,
            tc.tile_pool(name="impool", bufs=4) as impool,
            tc.tile_pool(name="wpool", bufs=4) as wpool,
            tc.tile_pool(name="thpool", bufs=4) as thpool,
            tc.tile_pool(name="scrpool", bufs=2) as scrpool,
            tc.tile_pool(name="spool", bufs=1) as spool,
            tc.tile_pool(name="pz", bufs=4, space="PSUM") as pzpool,
        ):
            # ---- constants: host packs bf16/fp16 already; scalar HW DGE ----
            ident16 = singles.tile([128, 128], BF16, tag="ident16")
            nc.scalar.dma_start(out=ident16[:], in_=ident_h[:])
            bd_sb = singles.tile([2 * KW, 98], BF16, tag="bd")
            nc.scalar.dma_start(out=bd_sb[:], in_=bd[:])
            cb_sb = singles.tile([98, 1], F32, tag="cb")
            nc.scalar.dma_start(out=cb_sb[:], in_=cb66[:])
            locq_sb = singles.tile([98, BL * A], BF16, tag="locq")
            nc.scalar.dma_start(out=locq_sb[:], in_=locq[:])
            vrep4 = singles.tile([128, 4 * A], F16, tag="vrep4")
            nc.scalar.dma_start(out=vrep4[:], in_=vrep[:])
            ident32 = singles.tile([128, 128], F32, tag="ident32")
            nc.scalar.dma_start(out=ident32[:], in_=ident[:])
            ident_r = singles.tile([128, 128], mybir.dt.float32r, tag="ident_r")
            nc.vector.tensor_copy(out=ident_r[:], in_=ident32[:])
            maskc_sb = singles.tile([128, 128], F32, tag="maskc")
            nc.sync.dma_start(out=maskc_sb[:], in_=maskc[:])
            prevc_sb = singles.tile([128, 128], F32, tag="prevc")
            nc.sync.dma_start(out=prevc_sb[:], in_=prevc[:])

            # ---- im2col on the sync HW-DGE queue (fp32) + DVE casts;
            #      keeps the software-DGE queue free for the w stream ----
            im_sb = []
            for g in range(NG):
                imf = impool.tile([2 * KW, TC], F32, tag="imf")
                base = prevp[2 * g : 2 * g + 2, :]
                imsrc = bass.AP(
                    tensor=base.tensor,
                    offset=base.offset,
                    ap=[[TP2, 2], [1, KW], [1, TC]],
                )
                nc.sync.dma_start(out=imf[:], in_=imsrc)
                im = impool.tile([2 * KW, TC], BF16, tag="im")
                im_sb.append(im)
                nc.vector.tensor_copy(out=im[:], in_=imf[:])

            # ---- conv phase: f[g] [98, TC]; rows 0-31 f(even batch),
            #      rows 32/33 ones, rows 64-95 f(odd), rows 96/97 ones ----
            f_sb = []
            for g in range(NG):
                fg = singles.tile([98, TC], BF16, tag=f"f{g}")
                f_sb.append(fg)
                im = im_sb[g]
                for t0, tsz in TR:
                    pc = pzpool.tile([128, 1024], F32, tag="z")
                    for u0 in (0, 512):
                        nc.tensor.matmul(
                            pc[0:98, u0 : u0 + 512],
                            bd_sb[:],
                            im[:, t0 + u0 : t0 + u0 + 512],
                            start=True,
                            stop=True,
                        )
                    # evacuate with conv bias; ones-rows get 0*psum + 1.0
                    nc.scalar.activation(
                        out=fg[:, t0 : t0 + tsz],
                        in_=pc[0:98, 0:tsz],
                        func=mybir.ActivationFunctionType.Identity,
                        bias=cb_sb[:, 0:1],
                        scale=1.0,
                    )

            # ---- main pass ----
            e_cols = spool.tile([128, 128], F32, tag="e_cols")
            for b in range(BL):
                if b % 2 == 0:
                    w_sb = wpool.tile([128, NCH * A], BF16, tag="w")
                    base = wmem[b // 2, :, :]
                    wid = ident16
                else:
                    w_sb = wpool.tile(
                        [128, NCH * A], mybir.dt.float32r, tag="w32"
                    )
                    base = wmem2[b // 2, :, :]
                    wid = ident_r
                wsrc = bass.AP(
                    tensor=base.tensor,
                    offset=base.offset,
                    ap=[[A, 128], [128 * A, NCH], [1, A]],
                )
                if b % 2 == 0:
                    nc.gpsimd.dma_start(out=w_sb[:], in_=wsrc)
                else:
                    nc.scalar.dma_start(out=w_sb[:], in_=wsrc)
                r0 = 64 * (b % 2)
                fg = f_sb[b // 2]
                for g in range(NG):
                    pzt = pzpool.tile([128, 1024], F32, tag="z")
                    for u0 in (0, 512):
                        nc.tensor.matmul(
                            pzt[:, u0 : u0 + 512],
                            wid[:],
                            w_sb[:, g * 1024 + u0 : g * 1024 + u0 + 512],
                            start=True,
                            stop=False,
                        )
                        for j in (u0 // A, u0 // A + 1):
                            ch = g * 4 + j
                            nc.tensor.matmul(
                                pzt[:, j * A : (j + 1) * A],
                                fg[r0 : r0 + 34, ch * 128 : (ch + 1) * 128],
                                locq_sb[r0 : r0 + 34, b * A : (b + 1) * A],
                                start=False,
                                stop=(j % 2 == 1),
                            )
                    th = thpool.tile([128, 1024], F16, tag="th")
                    nc.scalar.activation(
                        out=th[:],
                        in_=pzt[:],
                        func=mybir.ActivationFunctionType.Tanh,
                    )
                    col0 = b * NCH + g * 4
                    y = scrpool.tile([128, 1024], F16, tag="y")
                    nc.vector.tensor_mul(y[:], th[:], vrep4[:])
                    nc.vector.tensor_reduce(
                        out=e_cols[:, col0 : col0 + 4],
                        in_=y[:].rearrange("p (j a) -> p j a", j=4),
                        axis=mybir.AxisListType.X,
                        op=mybir.AluOpType.add,
                    )

            # ---- masked softmax in the column domain [toff, b*16+j] ----
            msk = spool.tile([128, 128], F32, tag="msk")
            nc.vector.tensor_mul(msk[:], e_cols[:], maskc_sb[:])
            m1 = spool.tile([128, 8], F32, tag="m1")
            nc.vector.tensor_reduce(
                out=m1[:],
                in_=msk[:].rearrange("p (b j) -> p b j", b=8),
                axis=mybir.AxisListType.X,
                op=mybir.AluOpType.max,
            )
            mx = spool.tile([128, 8], F32, tag="mx")
            nc.gpsimd.partition_all_reduce(
                mx[:], m1[:], 128, bass_isa.ReduceOp.max
            )
            mxr = spool.tile([128, 128], F32, tag="mxr")
            nc.vector.tensor_copy(
                out=mxr[:].rearrange("p (b j) -> p b j", b=8),
                in_=bass.AP(
                    tensor=mx.tensor,
                    offset=mx.offset,
                    ap=[[8, 128], [1, 8], [0, 16]],
                ),
            )
            sub = spool.tile([128, 128], F32, tag="sub")
            nc.vector.tensor_sub(sub[:], e_cols[:], mxr[:])
            ex = spool.tile([128, 128], F32, tag="ex")
            nc.scalar.activation(
                out=ex[:], in_=sub[:], func=mybir.ActivationFunctionType.Exp
            )
            num = spool.tile([128, 128], F32, tag="num")
            nc.vector.tensor_mul(num[:], ex[:], maskc_sb[:])
            s1 = spool.tile([128, 8], F32, tag="s1")
            nc.vector.tensor_reduce(
                out=s1[:],
                in_=num[:].rearrange("p (b j) -> p b j", b=8),
                axis=mybir.AxisListType.X,
                op=mybir.AluOpType.add,
            )
            ss = spool.tile([128, 8], F32, tag="ss")
            nc.gpsimd.partition_all_reduce(
                ss[:], s1[:], 128, bass_isa.ReduceOp.add
            )
            riv = spool.tile([128, 8], F32, tag="riv")
            nc.vector.reciprocal(riv[:], ss[:])
            rivr = spool.tile([128, 128], F32, tag="rivr")
            nc.vector.tensor_copy(
                out=rivr[:].rearrange("p (b j) -> p b j", b=8),
                in_=bass.AP(
                    tensor=riv.tensor,
                    offset=riv.offset,
                    ap=[[8, 128], [1, 8], [0, 16]],
                ),
            )
            ow_c = spool.tile([128, 128], F32, tag="ow_c")
            nc.vector.tensor_mul(ow_c[:], num[:], rivr[:])
            # transpose to [(b,j)-partitions, toff-free] and stream out
            pe_t = pzpool.tile([128, 1024], F32, tag="z")
            nc.tensor.matmul(
                pe_t[:, 0:128], ow_c[:], ident32[:], is_transpose=True,
                start=True, stop=True,
            )
            ow_t = spool.tile([128, 128], F32, tag="ow_t")
            nc.vector.tensor_copy(out=ow_t[:], in_=pe_t[:, 0:128])
            nw_t = spool.tile([128, 128], F32, tag="nw_t")
            nc.vector.tensor_add(nw_t[:], ow_t[:], prevc_sb[:])
            nc.sync.dma_start(out=out_w[:], in_=ow_t[:])
            nc.sync.dma_start(out=out_nw[:], in_=nw_t[:])

    nc.finalize()
    return nc


def make_in_maps(query, prev_weights, w_memory, memory_lengths, conv_w, conv_b,
                 loc_w, score_v, score_b):
    """Host-side prep (small params only) + batch sharding."""
    import ml_dtypes

    query = np.asarray(query, np.float32)
    prev_weights = np.asarray(prev_weights, np.float32)
    w_memory = np.asarray(w_memory, np.float32)
    memory_lengths = np.asarray(memory_lengths)
    conv_w = np.asarray(conv_w, np.float32)
    conv_b = np.asarray(conv_b, np.float32)
    loc_w = np.asarray(loc_w, np.float32)
    score_v = np.asarray(score_v, np.float32)
    score_b = np.asarray(score_b, np.float32)

    # block-diagonal conv kernel: even batch -> cols 0..31, odd -> 64..95;
    # cols 32/33/96/97 stay zero so the evac bias of 1.0 builds the
    # constant-1 rows used to fold in q (split into two bf16 rows).
    bd = np.zeros((2 * KW, 98), np.float32)
    bd[0:KW, 0:F] = conv_w[:, 0, :]
    bd[KW : 2 * KW, 64 : 64 + F] = conv_w[:, 0, :]
    cb66 = np.zeros((98, 1), np.float32)
    cb66[0:F, 0] = conv_b
    cb66[64 : 64 + F, 0] = conv_b
    cb66[32, 0] = 1.0
    cb66[33, 0] = 1.0
    cb66[96, 0] = 1.0
    cb66[97, 0] = 1.0
    vrep = np.tile(score_v[None, :], (128, 4)).astype(np.float16)
    ident = np.eye(128, dtype=np.float32)
    ident_h = np.eye(128, dtype=np.float32).astype(ml_dtypes.bfloat16)
    qb = query + score_b[None, :]  # [B, A]
    prevp_full = np.pad(prev_weights, ((0, 0), (PAD, TC - T + PAD)))

    def _bf16(x):
        u = x.astype(np.float32).view(np.uint32)
        u = (u + 0x8000 + ((u >> 16) & 1)) & 0xFFFF0000
        return u.view(np.float32)

    # column-domain t index: t = j*128 + toff
    tt = np.arange(NCH)[None, :] * 128 + np.arange(128)[:, None]  # [toff, j]

    in_maps = []
    for i in range(NCORES):
        s = slice(i * BL, (i + 1) * BL)
        wshard = np.zeros((BL, TC, A), np.float32)
        wshard[:, :T, :] = w_memory[s]
        wmem_even = np.ascontiguousarray(wshard[0::2])
        wmem_odd = np.ascontiguousarray(wshard[1::2])
        # moving operand mirrors the f-row structure (same base partition):
        # even batch reads rows 0..33, odd batch rows 64..97; q+score_b is
        # split into two bf16-exact rows (hi + residual) for full precision.
        locq_c = np.zeros((98, BL * A), np.float32)
        for b in range(BL):
            cs = slice(b * A, (b + 1) * A)
            r0 = 64 * (b % 2)
            q_hi = _bf16(qb[i * BL + b])
            q_lo = _bf16(qb[i * BL + b] - q_hi)
            locq_c[r0 : r0 + F, cs] = loc_w
            locq_c[r0 + 32, cs] = q_hi
            locq_c[r0 + 33, cs] = q_lo
        lens = memory_lengths[s]
        mask_c = np.zeros((128, 128), np.float32)
        prev_c = np.zeros((128, 128), np.float32)
        for b in range(BL):
            mask_c[:, b * NCH : (b + 1) * NCH] = (
                (tt < T) & (tt < lens[b])
            ).astype(np.float32)
            # prevc is consumed AFTER the transpose: [(b,j)-rows, toff]
            padded = np.pad(prev_weights[i * BL + b], (0, TC - T))
            prev_c[b * NCH : (b + 1) * NCH, :] = padded.reshape(NCH, 128)
        in_maps.append(
            {
                "wmem": wmem_even,
                "wmem2": wmem_odd,
                "prevp": np.ascontiguousarray(prevp_full[s]),
                "locq": locq_c.astype(ml_dtypes.bfloat16),
                "bd": bd.astype(ml_dtypes.bfloat16),
                "cb66": cb66,
                "vrep": vrep,
                "ident": ident,
                "ident_h": ident_h,
                "maskc": mask_c,
                "prevc": prev_c,
            }
        )
    return in_maps


_NC_CACHE = {}


def _get_nc():
    if "nc" not in _NC_CACHE:
        _NC_CACHE["nc"] = build_program()
    return _NC_CACHE["nc"]


def run(inputs, trace=False, tmpdir=None):
    """Run on 8 NeuronCores; returns ((output, new_weights), BassKernelResults)."""
    nc = _get_nc()
    in_maps = make_in_maps(**inputs)
    res = run_bass_kernel_spmd(
        nc, in_maps, core_ids=list(range(NCORES)), trace=trace, tmpdir=tmpdir
    )
    output = np.concatenate(
        [res.results[i]["out_w"][:, :T] for i in range(NCORES)], axis=0
    )
    new_w = np.concatenate(
        [res.results[i]["out_nw"][:, :T] for i in range(NCORES)], axis=0
    )
    return (output.astype(np.float32), new_w.astype(np.float32)), res


def kernel(**inputs):
    (output, new_w), _ = run(inputs, trace=False)
    return output, new_w


# revision 40
# speedup vs baseline: 1.4339x; 1.4339x over previous
"""Bahdanau attention cell (location-sensitive) on 8 TRN2 NeuronCores.

Sharding: data-parallel over the batch dim (64 -> 8 batches/core); all
params (conv kernel, location dense, score v/b) are tiny and replicated.

Per-core device program (Bass/Tile), [t-on-partitions, a-on-free] layout:
  1. conv(prev_weights) as a block-diagonal bf16 matmul over an im2col
     matrix (gpsimd cast-DMA); evacuation bias fabricates constant-1 rows
     so the location matmul can fold in query+score_b (split into two
     bf16-exact rows).
  2. main pass per (batch, 512-t group): PSUM[t, a] = ident16 @ w_bf16
     (w accumulate) + f-chunk.T @ [loc_w ; q_hi ; q_lo]; tanh on ACT ->
     fp16; DVE mult by score_v + segmented reduce -> e_cols[toff, b*16+j].
  3. softmax entirely in the column domain ([toff, (b,j)] tiles, tiny
     ops; cross-partition max/sum via gpsimd partition_all_reduce), one
     PE transpose, stream-matched DMAs straight to the padded outputs.
"""

import sys

sys.path.insert(0, "/opt/trn_rl_repo")

import numpy as np

import concourse.bacc as bacc
import concourse.bass as bass
import concourse.bass_isa as bass_isa
import concourse.tile as tile
from concourse import mybir
from concourse.bass_utils import run_bass_kernel_spmd

B, T, A, F, KW = 64, 2000, 256, 32, 31
NCORES = 8
BL = B // NCORES  # 8 batches per core
PAD = (KW - 1) // 2  # 15
TP = T + 2 * PAD
TC = 2048  # t padded to 16 chunks of 128
TP2 = TC + 2 * PAD  # padded prev: conv runs over all TC columns
NCH = TC // 128  # 16 chunks per batch
NG = 4  # groups of 4 chunks (512 t) per batch
F32 = mybir.dt.float32
F16 = mybir.dt.float16
BF16 = mybir.dt.bfloat16

TR = [(0, 1024), (1024, 1024)]
NGP_MOD = 0  # gpsimd cannot access PSUM on TRN2: keep w-add on PE


def build_program():
    nc = bacc.Bacc("TRN2", target_bir_lowering=False)

    wmem = nc.dram_tensor("wmem", [BL, TC, A], F32, kind="ExternalInput")
    prevp = nc.dram_tensor("prevp", [BL, TP2], F32, kind="ExternalInput")
    locq = nc.dram_tensor("locq", [98, BL * A], BF16, kind="ExternalInput")
    bd = nc.dram_tensor("bd", [2 * KW, 98], BF16, kind="ExternalInput")
    cb66 = nc.dram_tensor("cb66", [98, 1], F32, kind="ExternalInput")
    vrep = nc.dram_tensor("vrep", [128, 4 * A], F16, kind="ExternalInput")
    ident = nc.dram_tensor("ident", [128, 128], F32, kind="ExternalInput")
    ident_h = nc.dram_tensor("ident_h", [128, 128], BF16, kind="ExternalInput")
    maskc = nc.dram_tensor("maskc", [128, 128], F32, kind="ExternalInput")
    prevc = nc.dram_tensor("prevc", [128, 128], F32, kind="ExternalInput")
    out_w = nc.dram_tensor("out_w", [BL, TC], F32, kind="ExternalOutput")
    out_nw = nc.dram_tensor("out_nw", [BL, TC], F32, kind="ExternalOutput")

    with tile.TileContext(nc) as tc:
        with (
            tc.tile_pool(name="singles", bufs=1) as singles,
            tc.tile_pool(name="impool", bufs=4) as impool,
            tc.tile_pool(name="wpool", bufs=4) as wpool,
            tc.tile_pool(name="thpool", bufs=4) as thpool,
            tc.tile_pool(name="scrpool", bufs=2) as scrpool,
            tc.tile_pool(name="spool", bufs=1) as spool,
            tc.tile_pool(name="pz", bufs=4, space="PSUM") as pzpool,
        ):
            # ---- constants: host packs bf16/fp16 already; scalar HW DGE ----
            ident16 = singles.tile([128, 128], BF16, tag="ident16")
            nc.scalar.dma_start(out=ident16[:], in_=ident_h[:])
            bd_sb = singles.tile([2 * KW, 98], BF16, tag="bd")
            nc.scalar.dma_start(out=bd_sb[:], in_=bd[:])
            cb_sb = singles.tile([98, 1], F32, tag="cb")
            nc.scalar.dma_start(out=cb_sb[:], in_=cb66[:])
            locq_sb = singles.tile([98, BL * A], BF16, tag="locq")
            nc.scalar.dma_start(out=locq_sb[:], in_=locq[:])
            vrep4 = singles.tile([128, 4 * A], F16, tag="vrep4")
            nc.scalar.dma_start(out=vrep4[:], in_=vrep[:])
            ident32 = singles.tile([128, 128], F32, tag="ident32")
            nc.scalar.dma_start(out=ident32[:], in_=ident[:])
            maskc_sb = singles.tile([128, 128], F32, tag="maskc")
            nc.sync.dma_start(out=maskc_sb[:], in_=maskc[:])
            prevc_sb = singles.tile([128, 128], F32, tag="prevc")
            nc.sync.dma_start(out=prevc_sb[:], in_=prevc[:])

            # ---- im2col DMAs upfront (gpsimd cast-DMA fp32->bf16) ----
            im_sb = []
            for g in range(NG):
                im = impool.tile([2 * KW, TC], BF16, tag="im")
                im_sb.append(im)
                base = prevp[2 * g : 2 * g + 2, :]
                imsrc = bass.AP(
                    tensor=base.tensor,
                    offset=base.offset,
                    ap=[[TP2, 2], [1, KW], [1, TC]],
                )
                nc.gpsimd.dma_start(out=im[:], in_=imsrc)

            # ---- conv phase: f[g] [98, TC]; rows 0-31 f(even batch),
            #      rows 32/33 ones, rows 64-95 f(odd), rows 96/97 ones ----
            f_sb = []
            for g in range(NG):
                fg = singles.tile([98, TC], BF16, tag=f"f{g}")
                f_sb.append(fg)
                im = im_sb[g]
                for t0, tsz in TR:
                    pc = pzpool.tile([128, 1024], F32, tag="z")
                    for u0 in (0, 512):
                        nc.tensor.matmul(
                            pc[0:98, u0 : u0 + 512],
                            bd_sb[:],
                            im[:, t0 + u0 : t0 + u0 + 512],
                            start=True,
                            stop=True,
                        )
                    # evacuate with conv bias; ones-rows get 0*psum + 1.0
                    nc.scalar.activation(
                        out=fg[:, t0 : t0 + tsz],
                        in_=pc[0:98, 0:tsz],
                        func=mybir.ActivationFunctionType.Identity,
                        bias=cb_sb[:, 0:1],
                        scale=1.0,
                    )

            # ---- main pass ----
            e_cols = spool.tile([128, 128], F32, tag="e_cols")
            for b in range(BL):
                w_sb = wpool.tile([128, NCH * A], BF16, tag="w")
                base = wmem[b, :, :]
                wsrc = bass.AP(
                    tensor=base.tensor,
                    offset=base.offset,
                    ap=[[A, 128], [128 * A, NCH], [1, A]],
                )
                nc.gpsimd.dma_start(out=w_sb[:], in_=wsrc)
                r0 = 64 * (b % 2)
                fg = f_sb[b // 2]
                for g in range(NG):
                    pzt = pzpool.tile([128, 1024], F32, tag="z")
                    for u0 in (0, 512):
                        nc.tensor.matmul(
                            pzt[:, u0 : u0 + 512],
                            ident16[:],
                            w_sb[:, g * 1024 + u0 : g * 1024 + u0 + 512],
                            start=True,
                            stop=False,
                        )
                        for j in (u0 // A, u0 // A + 1):
                            ch = g * 4 + j
                            nc.tensor.matmul(
                                pzt[:, j * A : (j + 1) * A],
                                fg[r0 : r0 + 34, ch * 128 : (ch + 1) * 128],
                                locq_sb[r0 : r0 + 34, b * A : (b + 1) * A],
                                start=False,
                                stop=(j % 2 == 1),
                            )
                    th = thpool.tile([128, 1024], F16, tag="th")
                    nc.scalar.activation(
                        out=th[:],
                        in_=pzt[:],
                        func=mybir.ActivationFunctionType.Tanh,
                    )
                    col0 = b * NCH + g * 4
                    y = scrpool.tile([128, 1024], F16, tag="y")
                    nc.vector.tensor_mul(y[:], th[:], vrep4[:])
                    nc.vector.tensor_reduce(
                        out=e_cols[:, col0 : col0 + 4],
                        in_=y[:].rearrange("p (j a) -> p j a", j=4),
                        axis=mybir.AxisListType.X,
                        op=mybir.AluOpType.add,
                    )

            # ---- masked softmax in the column domain [toff, b*16+j] ----
            msk = spool.tile([128, 128], F32, tag="msk")
            nc.vector.tensor_mul(msk[:], e_cols[:], maskc_sb[:])
            m1 = spool.tile([128, 8], F32, tag="m1")
            nc.vector.tensor_reduce(
                out=m1[:],
                in_=msk[:].rearrange("p (b j) -> p b j", b=8),
                axis=mybir.AxisListType.X,
                op=mybir.AluOpType.max,
            )
            mx = spool.tile([128, 8], F32, tag="mx")
            nc.gpsimd.partition_all_reduce(
                mx[:], m1[:], 128, bass_isa.ReduceOp.max
            )
            mxr = spool.tile([128, 128], F32, tag="mxr")
            nc.vector.tensor_copy(
                out=mxr[:].rearrange("p (b j) -> p b j", b=8),
                in_=bass.AP(
                    tensor=mx.tensor,
                    offset=mx.offset,
                    ap=[[8, 128], [1, 8], [0, 16]],
                ),
            )
            sub = spool.tile([128, 128], F32, tag="sub")
            nc.vector.tensor_sub(sub[:], e_cols[:], mxr[:])
            ex = spool.tile([128, 128], F32, tag="ex")
            nc.scalar.activation(
                out=ex[:], in_=sub[:], func=mybir.ActivationFunctionType.Exp
            )
            num = spool.tile([128, 128], F32, tag="num")
            nc.vector.tensor_mul(num[:], ex[:], maskc_sb[:])
            s1 = spool.tile([128, 8], F32, tag="s1")
            nc.vector.tensor_reduce(
                out=s1[:],
                in_=num[:].rearrange("p (b j) -> p b j", b=8),
                axis=mybir.AxisListType.X,
                op=mybir.AluOpType.add,
            )
            ss = spool.tile([128, 8], F32, tag="ss")
            nc.gpsimd.partition_all_reduce(
                ss[:], s1[:], 128, bass_isa.ReduceOp.add
            )
            riv = spool.tile([128, 8], F32, tag="riv")
            nc.vector.reciprocal(riv[:], ss[:])
            rivr = spool.tile([128, 128], F32, tag="rivr")
            nc.vector.tensor_copy(
                out=rivr[:].rearrange("p (b j) -> p b j", b=8),
                in_=bass.AP(
                    tensor=riv.tensor,
                    offset=riv.offset,
                    ap=[[8, 128], [1, 8], [0, 16]],
                ),
            )
            ow_c = spool.tile([128, 128], F32, tag="ow_c")
            nc.vector.tensor_mul(ow_c[:], num[:], rivr[:])
            # transpose to [(b,j)-partitions, toff-free] and stream out
            pe_t = pzpool.tile([128, 1024], F32, tag="z")
            nc.tensor.matmul(
                pe_t[:, 0:128], ow_c[:], ident32[:], is_transpose=True,
                start=True, stop=True,
            )
            ow_t = spool.tile([128, 128], F32, tag="ow_t")
            nc.vector.tensor_copy(out=ow_t[:], in_=pe_t[:, 0:128])
            nw_t = spool.tile([128, 128], F32, tag="nw_t")
            nc.vector.tensor_add(nw_t[:], ow_t[:], prevc_sb[:])
            nc.sync.dma_start(out=out_w[:], in_=ow_t[:])
            nc.sync.dma_start(out=out_nw[:], in_=nw_t[:])

    nc.finalize()
    return nc


def make_in_maps(query, prev_weights, w_memory, memory_lengths, conv_w, conv_b,
                 loc_w, score_v, score_b):
    """Host-side prep (small params only) + batch sharding."""
    import ml_dtypes

    query = np.asarray(query, np.float32)
    prev_weights = np.asarray(prev_weights, np.float32)
    w_memory = np.asarray(w_memory, np.float32)
    memory_lengths = np.asarray(memory_lengths)
    conv_w = np.asarray(conv_w, np.float32)
    conv_b = np.asarray(conv_b, np.float32)
    loc_w = np.asarray(loc_w, np.float32)
    score_v = np.asarray(score_v, np.float32)
    score_b = np.asarray(score_b, np.float32)

    # block-diagonal conv kernel: even batch -> cols 0..31, odd -> 64..95;
    # cols 32/33/96/97 stay zero so the evac bias of 1.0 builds the
    # constant-1 rows used to fold in q (split into two bf16 rows).
    bd = np.zeros((2 * KW, 98), np.float32)
    bd[0:KW, 0:F] = conv_w[:, 0, :]
    bd[KW : 2 * KW, 64 : 64 + F] = conv_w[:, 0, :]
    cb66 = np.zeros((98, 1), np.float32)
    cb66[0:F, 0] = conv_b
    cb66[64 : 64 + F, 0] = conv_b
    cb66[32, 0] = 1.0
    cb66[33, 0] = 1.0
    cb66[96, 0] = 1.0
    cb66[97, 0] = 1.0
    vrep = np.tile(score_v[None, :], (128, 4)).astype(np.float16)
    ident = np.eye(128, dtype=np.float32)
    ident_h = np.eye(128, dtype=np.float32).astype(ml_dtypes.bfloat16)
    qb = query + score_b[None, :]  # [B, A]
    prevp_full = np.pad(prev_weights, ((0, 0), (PAD, TC - T + PAD)))

    def _bf16(x):
        u = x.astype(np.float32).view(np.uint32)
        u = (u + 0x8000 + ((u >> 16) & 1)) & 0xFFFF0000
        return u.view(np.float32)

    # column-domain t index: t = j*128 + toff
    tt = np.arange(NCH)[None, :] * 128 + np.arange(128)[:, None]  # [toff, j]

    in_maps = []
    for i in range(NCORES):
        s = slice(i * BL, (i + 1) * BL)
        wshard = np.zeros((BL, TC, A), np.float32)
        wshard[:, :T, :] = w_memory[s]
        # moving operand mirrors the f-row structure (same base partition):
        # even batch reads rows 0..33, odd batch rows 64..97; q+score_b is
        # split into two bf16-exact rows (hi + residual) for full precision.
        locq_c = np.zeros((98, BL * A), np.float32)
        for b in range(BL):
            cs = slice(b * A, (b + 1) * A)
            r0 = 64 * (b % 2)
            q_hi = _bf16(qb[i * BL + b])
            q_lo = _bf16(qb[i * BL + b] - q_hi)
            locq_c[r0 : r0 + F, cs] = loc_w
            locq_c[r0 + 32, cs] = q_hi
            locq_c[r0 + 33, cs] = q_lo
        lens = memory_lengths[s]
        mask_c = np.zeros((128, 128), np.float32)
        prev_c = np.zeros((128, 128), np.float32)
        for b in range(BL):
            mask_c[:, b * NCH : (b + 1) * NCH] = (
                (tt < T) & (tt < lens[b])
            ).astype(np.float32)
            # prevc is consumed AFTER the transpose: [(b,j)-rows, toff]
            padded = np.pad(prev_weights[i * BL + b], (0, TC - T))
            prev_c[b * NCH : (b + 1) * NCH, :] = padded.reshape(NCH, 128)
        in_maps.append(
            {
                "wmem": wshard,
                "prevp": np.ascontiguousarray(prevp_full[s]),
                "locq": locq_c.astype(ml_dtypes.bfloat16),
                "bd": bd.astype(ml_dtypes.bfloat16),
                "cb66": cb66,
                "vrep": vrep,
                "ident": ident,
                "ident_h": ident_h,
                "maskc": mask_c,
                "prevc": prev_c,
            }
        )
    return in_maps


_NC_CACHE = {}


def _get_nc():
    if "nc" not in _NC_CACHE:
        _NC_CACHE["nc"] = build_program()
    return _NC_CACHE["nc"]


def run(inputs, trace=False, tmpdir=None):
    """Run on 8 NeuronCores; returns ((output, new_weights), BassKernelResults)."""
    nc = _get_nc()
    in_maps = make_in_maps(**inputs)
    res = run_bass_kernel_spmd(
        nc, in_maps, core_ids=list(range(NCORES)), trace=trace, tmpdir=tmpdir
    )
    output = np.concatenate(
        [res.results[i]["out_w"][:, :T] for i in range(NCORES)], axis=0
    )
    new_w = np.concatenate(
        [res.results[i]["out_nw"][:, :T] for i in range(NCORES)], axis=0
    )
    return (output.astype(np.float32), new_w.astype(np.float32)), res


def kernel(**inputs):
    (output, new_w), _ = run(inputs, trace=False)
    return output, new_w
